# revision 4
# baseline (speedup 1.0000x reference)
"""Haar DWT on 8 Trainium2 NeuronCores (batch-parallel, 1 image per core).

Layout: partition p of tile t holds 16 consecutive input rows (8 output
rows) of one channel: global row-block g = 128*t + p, channel c = g//32,
rows 16*(g%32)..+16. Free dim = 8192 (16 rows x 512 cols) = 32 KiB, so
every load descriptor is one 32 KiB contiguous DRAM span per partition
and every store descriptor is 8 KiB (vs 16/4 KiB at 8-row tiles; the
~43 ns fixed cost per descriptor drops from 7%/29% of line-rate time to
3.4%/12.5%).

Per-core pipeline, 16 tiles (2 channels each):
  1. in-DMA: 4 MiB fully contiguous (SP HWDGE ring)
  2. DVE stage 1 (column butterfly, stride-2 views, two 4096-FD halves
     so the sd scratch stays at 16 KiB/partition): the Haar 0.5 scale is
     folded in via the stock LN_BWD_DX_ANT custom-DVE op
     (Src0 - Src1*C0 - C1)*C2:
       sum1 = 0.5*(x[0::2] + x[1::2])   (C0=-1, C2=+0.5)
       diff1 = 0.5*(x[1::2] - x[0::2])  (C0=+1, C2=-0.5)
     -> no ScalarE pass at all; ACT only dispatches stores.
  3. DVE stage 2 (row butterfly, 3-dim APs):
       add -> LL (from sum) + HL (from diff); sub -> LH + HH
     o_sb layout [sb:4][j:8][w:256]: per partition each subband block is
     8 KiB = 8 consecutive output rows, contiguous in DRAM
  4. two 2 MiB out-DMAs per tile ({LL,HL} after the adds, {LH,HH} after
     the subs) on the ACT HWDGE ring so stores do not serialize behind
     loads on Q-SP

No PE/PSUM, ScalarE idle, DVE makes the two minimal butterfly passes.
"""

import sys

sys.path.insert(0, "/opt/trn_rl_repo")

import numpy as np

import concourse.bass as bass
import concourse.bacc as bacc
import concourse.mybir as mybir
from concourse import tile
from concourse.bass_utils import run_bass_kernel_spmd

N_CORES = 8
C = 64
H = 512
W = 512
HO = H // 2
WO = W // 2
P = 128
FD = 8192               # 16 input rows per partition
TILES = C * H * W // (P * FD)  # 16
HFD = FD // 2           # 4096: one 8-row half
OFD = FD // 4           # 2048: out elems per partition per subband

F32 = mybir.dt.float32


def build_nc() -> bass.Bass:
    nc = bacc.Bacc()
    x = nc.dram_tensor("x", [C, H, W], F32, kind="ExternalInput")
    out = nc.dram_tensor("out", [4 * C, HO, WO], F32, kind="ExternalOutput")

    # [2048 row-blocks, 8192]: row-block g = (c, hb), free = (r, w), h = 16*hb + r
    x_v = x.rearrange("c (hb r) w -> (c hb) (r w)", r=16)
    # per subband: out[sb*64 + cc, h, w] flattened — offset = g*2048 + j*256 + w
    out_v = out.rearrange("(s cc) h w -> s (cc h w)", s=4)

    with tile.TileContext(nc) as tc:
        with (
            tc.tile_pool(name="pin", bufs=3) as pin,
            tc.tile_pool(name="psd", bufs=2) as psd,
            tc.tile_pool(name="pout", bufs=2) as pout,
        ):
            for t in range(TILES):
                in_sb = pin.tile([P, FD], F32)
                nc.sync.dma_start(in_sb[:], x_v[t * P : (t + 1) * P, :])

                o_sb = pout.tile([P, FD], F32)
                # o_sb: [sb:4][j:8][w:256]
                o4 = o_sb[:].rearrange("p (sb j w) -> p sb j w", sb=4, j=8)

                sds = []
                for h in range(2):
                    # column butterfly on 8-row half h, 0.5 folded in:
                    #   ln_bwd_dx computes (dy - x_hat*mean_dyx - mean_dy)*scale
                    sd = psd.tile([P, HFD], F32)
                    i3 = in_sb[:, h * HFD : (h + 1) * HFD].rearrange(
                        "p (k two) -> p k two", two=2
                    )
                    nc.vector.ln_bwd_dx(
                        sd[:, 0:2048], i3[:, :, 0], i3[:, :, 1], -1.0, 0.0, 0.5
                    )
                    nc.vector.ln_bwd_dx(
                        sd[:, 2048:4096], i3[:, :, 0], i3[:, :, 1], 1.0, 0.0, -0.5
                    )
                    sds.append(sd)

                # stage 2 split across engines so neither approaches the DMA
                # roofline: adds (LL/HL) on GpSimd (idle otherwise; fp32 add
                # at 0.42 eff ≈ 4 µs/op, launch 95 ns), subs (LH/HH) on DVE.
                # DVE 2-input fp32 ops run 1× on the dedicated port pair, so
                # the two engines don't contend for SBUF ports.
                s4s = [
                    sd[:].rearrange(
                        "p (half j parity w) -> p half j parity w",
                        half=2, j=4, parity=2,
                    )
                    for sd in sds
                ]
                for h, s4 in enumerate(s4s):
                    oh = o4[:, :, 4 * h : 4 * h + 4, :]
                    # LL (sb0) from sum-half, HL (sb2) from diff-half
                    nc.gpsimd.tensor_add(
                        oh[:, 0::2, :, :], s4[:, :, :, 0, :], s4[:, :, :, 1, :]
                    )
                for h, s4 in enumerate(s4s):
                    oh = o4[:, :, 4 * h : 4 * h + 4, :]
                    # LH (sb1) from sum-half, HH (sb3) from diff-half
                    nc.vector.tensor_sub(
                        oh[:, 1::2, :, :], s4[:, :, :, 1, :], s4[:, :, :, 0, :]
                    )

                # two 2 MiB stores per tile (ACT ring): {LL,HL} can go as
                # soon as the adds land, {LH,HH} after the subs.
                dst = out_v[:, t * P * OFD : (t + 1) * P * OFD].rearrange(
                    "s (p f) -> p s f", f=OFD
                )
                src4 = o_sb[:].rearrange("p (s f) -> p s f", s=4)
                nc.scalar.dma_start(dst[:, 0::2, :], src4[:, 0::2, :])
                nc.scalar.dma_start(dst[:, 1::2, :], src4[:, 1::2, :])

    nc.finalize()
    return nc


_NC_CACHE: dict = {}


def _get_nc() -> bass.Bass:
    if "nc" not in _NC_CACHE:
        _NC_CACHE["nc"] = build_nc()
    return _NC_CACHE["nc"]


def kernel(x: np.ndarray) -> np.ndarray:
    x = np.asarray(x)
    assert x.shape == (N_CORES, C, H, W), x.shape
    nc = _get_nc()
    in_maps = [{"x": np.ascontiguousarray(x[i])} for i in range(N_CORES)]
    res = run_bass_kernel_spmd(nc, in_maps, list(range(N_CORES)))
    return np.stack([res.results[i]["out"] for i in range(N_CORES)], axis=0)


# revision 7
# speedup vs baseline: 1.1890x; 1.1890x over previous
"""Haar DWT on 8 Trainium2 NeuronCores (batch-parallel, 1 image per core).

Layout: partition p of tile t holds 16 consecutive input rows (8 output
rows) of one channel: global row-block g = 128*t + p, channel c = g//32,
rows 16*(g%32)..+16. Free dim = 8192 (16 rows x 512 cols) = 32 KiB, so
every load descriptor is one 32 KiB contiguous DRAM span per partition
and every store descriptor is 8 KiB (the ~43 ns fixed cost per
descriptor is 3.4%/12.5% of line-rate time vs 7%/29% at 8-row tiles).

Per-core pipeline, 16 tiles (2 channels each):
  1. in-DMA: 4 MiB fully contiguous (SP HWDGE ring)
  2. DVE stage 1 (column butterfly, stride-2 views, two 4096-FD halves
     so the sd scratch stays at 16 KiB/partition): the Haar 0.5 scale is
     folded in via the stock LN_BWD_DX_ANT custom-DVE op
     (Src0 - Src1*C0 - C1)*C2:
       sum1 = 0.5*(x[0::2] + x[1::2])   (C0=-1, C2=+0.5)
       diff1 = 0.5*(x[1::2] - x[0::2])  (C0=+1, C2=-0.5)
     -> no ScalarE pass at all; ACT only dispatches stores.
  3. DVE stage 2 (row butterfly, 3-dim APs):
       add -> LL (from sum) + HL (from diff); sub -> LH + HH
     o_sb layout [sb:4][j:8][w:256]: per partition each subband block is
     8 KiB = 8 consecutive output rows, contiguous in DRAM.
     All compute stays on DVE: GpSimd shares DVE's second SBUF port
     pair, and concurrent Pool tensor ops measurably stall DVE's
     2-stream fp32 ops (LN_BWD_DX 2.2->3.2us, SUB 2.2->4.5us) — tried
     and reverted. ACT can't help (bias must be [P,1]).
  4. two 2 MiB out-DMAs per tile ({LL,HL} after the adds, {LH,HH} after
     the subs) on the ACT HWDGE ring so stores do not serialize behind
     loads on Q-SP.
  5. The LAST tile runs as 4 independent quarter-chunks (own 1 MiB load,
     own stores) so the final DVE chain is ~4 us instead of ~17 us —
     shrinks the tail drain where the store ring idles waiting on DVE.

No PE/PSUM, ScalarE idle, DVE makes the two minimal butterfly passes.
"""

import sys

sys.path.insert(0, "/opt/trn_rl_repo")

import numpy as np

import concourse.bass as bass
import concourse.bacc as bacc
import concourse.mybir as mybir
from concourse import tile
from concourse.bass_utils import run_bass_kernel_spmd

N_CORES = 8
C = 64
H = 512
W = 512
HO = H // 2
WO = W // 2
P = 128
FD = 8192               # 16 input rows per partition
TILES = C * H * W // (P * FD)  # 16
OFD = FD // 4           # 2048: out elems per partition per subband

F32 = mybir.dt.float32


def build_nc() -> bass.Bass:
    nc = bacc.Bacc()
    x = nc.dram_tensor("x", [C, H, W], F32, kind="ExternalInput")
    out = nc.dram_tensor("out", [4 * C, HO, WO], F32, kind="ExternalOutput")

    # [2048 row-blocks, 8192]: row-block g = (c, hb), free = (r, w), h = 16*hb + r
    x_v = x.rearrange("c (hb r) w -> (c hb) (r w)", r=16)
    # per subband: out[sb*64 + cc, h, w] flattened — offset = g*2048 + j*256 + w
    out_v = out.rearrange("(s cc) h w -> s (cc h w)", s=4)

    with tile.TileContext(nc) as tc:
        with (
            tc.tile_pool(name="pin", bufs=3) as pin,
            tc.tile_pool(name="psd", bufs=2) as psd,
            tc.tile_pool(name="pout", bufs=2) as pout,
        ):

            def chunk(t: int, f0: int, flen: int):
                """Process free-dim slice [f0, f0+flen) of tile t:
                load, butterfly both stages, store. flen % 2048 == 0."""
                in_sb = pin.tile([P, flen], F32, name="in_sb")
                nc.sync.dma_start(
                    in_sb[:], x_v[t * P : (t + 1) * P, f0 : f0 + flen]
                )

                o_sb = pout.tile([P, flen], F32, name="o_sb")
                nj = flen // 1024  # output rows in this chunk per partition
                # o_sb: [sb:4][j:nj][w:256]
                o4 = o_sb[:].rearrange("p (sb j w) -> p sb j w", sb=4, j=nj)

                # stage 1 in <=4096-FD pieces so sd scratch stays <=16 KiB
                npc = max(1, flen // 4096)     # pieces
                plen = flen // npc             # 4096 (or 2048 for quarters)
                sds = []
                for h in range(npc):
                    sd = psd.tile([P, plen], F32, name="sd")
                    i3 = in_sb[:, h * plen : (h + 1) * plen].rearrange(
                        "p (k two) -> p k two", two=2
                    )
                    hp = plen // 2
                    nc.vector.ln_bwd_dx(
                        sd[:, 0:hp], i3[:, :, 0], i3[:, :, 1], -1.0, 0.0, 0.5
                    )
                    nc.vector.ln_bwd_dx(
                        sd[:, hp:plen], i3[:, :, 0], i3[:, :, 1], 1.0, 0.0, -0.5
                    )
                    sds.append(sd)

                # stage 2: adds for all pieces first so the {LL,HL} store
                # fires early; subs then release the {LH,HH} store.
                pj = plen // 1024  # output rows per piece
                s4s = [
                    sd[:].rearrange(
                        "p (half j parity w) -> p half j parity w",
                        half=2, j=pj, parity=2,
                    )
                    for sd in sds
                ]
                for h, s4 in enumerate(s4s):
                    oh = o4[:, :, pj * h : pj * h + pj, :]
                    nc.vector.tensor_add(
                        oh[:, 0::2, :, :], s4[:, :, :, 0, :], s4[:, :, :, 1, :]
                    )
                for h, s4 in enumerate(s4s):
                    oh = o4[:, :, pj * h : pj * h + pj, :]
                    nc.vector.tensor_sub(
                        oh[:, 1::2, :, :], s4[:, :, :, 1, :], s4[:, :, :, 0, :]
                    )

                # stores on the ACT ring
                dst = out_v[:, t * P * OFD : (t + 1) * P * OFD].rearrange(
                    "s (p f) -> p s f", f=OFD
                )[:, :, f0 // 4 : (f0 + flen) // 4]
                src4 = o_sb[:].rearrange("p (s f) -> p s f", s=4)
                nc.scalar.dma_start(dst[:, 0::2, :], src4[:, 0::2, :])
                nc.scalar.dma_start(dst[:, 1::2, :], src4[:, 1::2, :])

            for t in range(TILES - 1):
                chunk(t, 0, FD)
            # last tile as 4 quarter-chunks to shrink the tail drain
            for q in range(4):
                chunk(TILES - 1, q * (FD // 4), FD // 4)

    nc.finalize()
    return nc


_NC_CACHE: dict = {}


def _get_nc() -> bass.Bass:
    if "nc" not in _NC_CACHE:
        _NC_CACHE["nc"] = build_nc()
    return _NC_CACHE["nc"]


def kernel(x: np.ndarray) -> np.ndarray:
    x = np.asarray(x)
    assert x.shape == (N_CORES, C, H, W), x.shape
    nc = _get_nc()
    in_maps = [{"x": np.ascontiguousarray(x[i])} for i in range(N_CORES)]
    res = run_bass_kernel_spmd(nc, in_maps, list(range(N_CORES)))
    return np.stack([res.results[i]["out"] for i in range(N_CORES)], axis=0)
